# revision 13
# baseline (speedup 1.0000x reference)
"""DRR (digitally reconstructed radiograph) kernel for 8 Trainium2 cores.

Strategy: the cone-beam geometry is separable — per batch the source is a
single point and detector targets form an axis-aligned grid, so for each
ray-sample index s every ray lies in the same z-plane, with x depending only
on the detector column and y only on the detector row.  Trilinear
interpolation of the whole detector at sample s therefore factors into
   img_s = V_s^T @ [(1-wz)*A_{k0} + wz*A_{k0+1}] @ U_s
where A_k = density[:, :, k] and U_s / V_s are 256x256 "hat" interpolation
matrices (two nonzeros per column).  Samples whose z-plane misses the volume
contribute exactly zero and are skipped.  The ~65 surviving (batch, s) pairs
are sharded across the 8 cores (cores 0-3 batch 0, cores 4-7 batch 1).

v3: everything the device consumes is packed on the host into one fp8 blob
per core ([128, NP*1536]): per sample the z-blended slab (mean-centered, the
0.5 offset restored exactly on device via a rank-K bias matmul since hat
columns sum to exactly 1 in-volume / 0 outside) and the U|V hat matrices
with interpolation fractions snapped to a 1/16 grid (so both hat taps are
exactly representable in fp8e4m3 and quantization acts as <=1/32-voxel
coordinate jitter instead of breaking the partition of unity).  The blob is
fetched with 3 big chunked DMAs (minimizing sequencer DIRECT2D dispatch and
semaphore traffic that dominated earlier revisions), and the device runs a
nearly pure TensorE stream: per sample 4 matmuls (slab^T @ U), one
PSUM->SBUF bf16 copy, 4 matmuls (V^T @ o1) into persistent PSUM images,
plus a final raylen/n_points scale (precomputed on host).
"""

import numpy as np

DV = 256
H = W = 256

_PROGRAM_CACHE = {}


def _build_program(NP):
    """Per-core Bass/Tile program for NP (batch,sample) pair slots."""
    import concourse.bass as bass
    import concourse.mybir as mybir
    from concourse import tile
    from concourse import bacc

    dt = mybir.dt
    F32, BF16, F8 = dt.float32, dt.bfloat16, dt.float8e4
    COPY = mybir.ActivationFunctionType.Copy
    MUL = mybir.AluOpType.mult

    # chunk boundaries for the blob DMA: small leading chunks so the PE can
    # start as soon as possible, larger trailing chunks for DMA efficiency
    cuts = sorted({0, min(1, NP), min(3, NP), min(6, NP), NP})
    chunks = [(lo, hi) for lo, hi in zip(cuts[:-1], cuts[1:]) if hi > lo]

    nc = bacc.Bacc()
    blobd = nc.declare_dram_parameter("blob", [128, NP * 1536], F8, isOutput=False)
    biasd = nc.declare_dram_parameter("bias", [2, 8 * 512], BF16, isOutput=False)
    rsd = nc.declare_dram_parameter("rs", [128, 512], F32, isOutput=False)
    partial = nc.declare_dram_parameter("partial", [128, 512], F32, isOutput=True)

    with tile.TileContext(nc) as tc:
        with (
            tc.tile_pool(name="const", bufs=1) as cpool,
            tc.tile_pool(name="blob", bufs=1) as bpool,
            tc.tile_pool(name="o1", bufs=3) as o1pool,
            tc.tile_pool(name="ps1", bufs=3, space=bass.MemorySpace.PSUM) as ps1,
            tc.tile_pool(name="psimg", bufs=1, space=bass.MemorySpace.PSUM) as psimg,
        ):
            blob = {}
            for ci, (lo, hi) in enumerate(chunks):
                t = bpool.tile([128, (hi - lo) * 1536], F8,
                               name=f"blob{ci}", tag=f"blob{ci}")
                nc.sync.dma_start(t[:], blobd[:, lo * 1536:hi * 1536])
                blob[ci] = (t, lo)

            bias = cpool.tile([2, 8 * 512], BF16, name="bias", tag="bias")
            rs = cpool.tile([128, 512], F32, name="rs", tag="rs")
            nc.scalar.dma_start(bias[:], biasd[:, :])
            nc.scalar.dma_start(rs[:], rsd[:, :])

            # persistent image accumulators — one PSUM bank per accumulation
            # group (start=True clears has_written at bank granularity, so
            # two open groups must not share a bank)
            img = [psimg.tile([128, 256], F32, name=f"img{ht}", tag=f"img{ht}")
                   for ht in range(2)]

            # rank-16 bias: img[h, w] += sum_r Vb[r, h] * Ub[r, w]
            # (restores the 0.5 slab offset; unused rows are zero).
            # Split into 8 K=2 slices (pairs of rows live side by side in the
            # free dim so every slice sits at partition 0): same math, but the
            # extra matmuls run during the blob-DMA wait and warm the PE HAM
            # clock gate.
            for ht in range(2):
                for k in range(8):
                    nc.tensor.matmul(
                        img[ht][:],
                        bias[:, k * 512 + ht * 128:k * 512 + (ht + 1) * 128],
                        bias[:, k * 512 + 256:k * 512 + 512],
                        start=(k == 0),
                        stop=False,
                    )

            for ci, (lo, hi) in enumerate(chunks):
                t, base_i = blob[ci]
                for i in range(lo, hi):
                    off = (i - base_i) * 1536
                    # mm1: o1[y, w] = sum_x slabC[x, y] * U[x, w]
                    p1 = ps1.tile([128, 512], F32, name="p1", tag="p1")
                    for yh in range(2):
                        for xh in range(2):
                            nc.tensor.matmul(
                                p1[:, yh * 256:(yh + 1) * 256],
                                t[:, off + xh * 256 + yh * 128:
                                   off + xh * 256 + yh * 128 + 128],
                                t[:, off + 512 + xh * 256:
                                   off + 512 + (xh + 1) * 256],
                                start=(xh == 0),
                                stop=(xh == 1),
                            )
                    o1 = o1pool.tile([128, 512], BF16, name="o1", tag="o1")
                    nc.scalar.activation(o1[:], p1[:], COPY)

                    # mm2: img[h, w] += sum_y V[y, h] * o1[y, w]
                    for ht in range(2):
                        for yh in range(2):
                            nc.tensor.matmul(
                                img[ht][:],
                                t[:, off + 1024 + yh * 256 + ht * 128:
                                   off + 1024 + yh * 256 + ht * 128 + 128],
                                o1[:, yh * 256:(yh + 1) * 256],
                                start=False,
                                stop=(i == NP - 1 and yh == 1),
                            )

            # epilogue: scale by raylen/S and store
            fin = cpool.tile([128, 512], F32, name="fin", tag="fin")
            for ht in range(2):
                nc.vector.tensor_tensor(fin[:, ht * 256:(ht + 1) * 256],
                                        img[ht][:],
                                        rs[:, ht * 256:(ht + 1) * 256], MUL)
            nc.sync.dma_start(partial[:, :], fin[:])

    nc.compile()
    return nc


def _np_reference(source, target, density, spacing, origin, n_points):
    """Pure-numpy fallback mirroring the reference exactly (only used if the
    inputs lack the separable cone-beam structure)."""
    B = source.shape[0]
    S = int(n_points)
    t = np.linspace(0.0, 1.0, S, dtype=np.float32)
    ray = (target - source).astype(np.float32)
    pts = source[:, :, None, :] + t[None, None, :, None] * ray[:, :, None, :]
    idx = ((pts - origin) / spacing).astype(np.float32)
    f = np.floor(idx)
    w = idx - f
    fi = f.astype(np.int32)
    hi = np.array([DV - 1] * 3, np.float32)
    inside = np.all((idx >= 0) & (idx <= hi), axis=-1)
    wx, wy, wz = w[..., 0], w[..., 1], w[..., 2]
    out = np.zeros(idx.shape[:-1], np.float32)
    for di in (0, 1):
        for dj in (0, 1):
            for dk in (0, 1):
                ci = np.clip(fi[..., 0] + di, 0, DV - 1)
                cj = np.clip(fi[..., 1] + dj, 0, DV - 1)
                ck = np.clip(fi[..., 2] + dk, 0, DV - 1)
                wgt = ((wx if di else 1.0 - wx) * (wy if dj else 1.0 - wy)
                       * (wz if dk else 1.0 - wz)).astype(np.float32)
                out = out + density[ci, cj, ck] * wgt
    out = out * inside
    raylen = np.sqrt((ray * ray).sum(-1))
    img = out.sum(-1) * raylen / np.float32(S)
    return img.reshape(B, 1, H, W)


def _plan_pairs(source, target, spacing, origin, S):
    """Per batch: list of (s, k0, k1, wz, X[256], Y[256]) for in-volume
    samples, mirroring the reference's f32 arithmetic."""
    B = source.shape[0]
    T = target.reshape(B, H, W, 3)
    src = source[:, 0, :]
    t = np.linspace(0.0, 1.0, S, dtype=np.float32)
    plans = []
    for b in range(B):
        x_w = T[b, 0, :, 0]
        y_h = T[b, :, 0, 1]
        z_c = T[b, 0, 0, 2]
        lst = []
        for s in range(S):
            zc = ((src[b, 2] + np.float32(t[s] * (z_c - src[b, 2])))
                  - origin[2]) / spacing[2]
            if not (0.0 <= zc <= DV - 1):
                continue
            k0 = int(np.floor(zc))
            wz = np.float32(zc - k0)
            k1 = min(k0 + 1, DV - 1)
            X = ((src[b, 0] + (t[s] * (x_w - src[b, 0])).astype(np.float32))
                 - origin[0]) / spacing[0]
            Y = ((src[b, 1] + (t[s] * (y_h - src[b, 1])).astype(np.float32))
                 - origin[1]) / spacing[1]
            X = np.where((X >= 0) & (X <= DV - 1), X, np.float32(-10.0))
            Y = np.where((Y >= 0) & (Y <= DV - 1), Y, np.float32(-10.0))
            lst.append((s, k0, k1, wz, X.astype(np.float32), Y.astype(np.float32)))
        plans.append(lst)
    return plans


def _hat_grid16(X):
    """Dense hat matrix with fractions snapped to a 1/16 grid so that both
    taps (f, 1-f) are exactly representable in fp8e4m3.  [x, w], f32."""
    Xq = np.where(X < 0, X,
                  np.floor(X) + np.round((X - np.floor(X)) * 16.0) / np.float32(16.0))
    x = np.arange(DV, dtype=np.float32)[:, None]
    return np.maximum(np.float32(0.0),
                      1.0 - np.abs(Xq[None, :] - x)).astype(np.float32)


def _pack(m):
    """[256, N] -> [128, 2*N] with free = half*N + col, partition = row%128."""
    n = m.shape[1]
    return m.reshape(2, 128, n).transpose(1, 0, 2).reshape(128, 2 * n)


def kernel(source, target, density, spacing, origin, n_points):
    import ml_dtypes
    from concourse.bass_utils import run_bass_kernel_spmd

    source = np.asarray(source, np.float32)
    target = np.asarray(target, np.float32)
    density = np.asarray(density, np.float32)
    spacing = np.asarray(spacing, np.float32)
    origin = np.asarray(origin, np.float32)
    S = int(n_points)
    B = source.shape[0]

    # separability preconditions for the fast path
    T = target.reshape(B, H, W, 3)
    sep = (
        B == 2 and S >= 2 and density.shape == (DV, DV, DV)
        and np.all(source == source[:, :1, :])
        and np.all(T[..., 0] == T[:, :1, :, 0])
        and np.all(T[..., 1] == T[:, :, :1, 1])
        and np.all(T[..., 2] == T[:, :1, :1, 2])
    )
    if not sep:
        return _np_reference(source, target, density, spacing, origin, S)

    plans = _plan_pairs(source, target, spacing, origin, S)

    # shard: cores 0-3 -> batch 0, cores 4-7 -> batch 1 (B == 2)
    core_batch = [0, 0, 0, 0, 1, 1, 1, 1]
    core_pairs = [[] for _ in range(8)]
    for b in range(2):
        cores = [c for c in range(8) if core_batch[c] == b]
        for n, pair in enumerate(plans[b]):
            core_pairs[cores[n % len(cores)]].append(pair)
    NP = max(1, max(len(p) for p in core_pairs))

    nc = _PROGRAM_CACHE.get(NP)
    if nc is None:
        nc = _build_program(NP)
        _PROGRAM_CACHE[NP] = nc

    F8 = ml_dtypes.float8_e4m3fn
    BF = ml_dtypes.bfloat16
    in_maps = []
    for c in range(8):
        b = core_batch[c]
        pairs = core_pairs[c]
        blob = np.zeros((128, NP * 1536), F8)
        bias = np.zeros((2, 8, 512), BF)
        for n, (s, k0, k1, wz, X, Y) in enumerate(pairs):
            off = n * 1536
            # z-blend in f32, center at 0, quantize to fp8
            arr = (density[:, :, k0] * (1.0 - wz) + density[:, :, k1] * wz
                   - np.float32(0.5))
            blob[:, off:off + 512] = _pack(arr).astype(F8)
            blob[:, off + 512:off + 1024] = _pack(_hat_grid16(X)).astype(F8)
            blob[:, off + 1024:off + 1536] = _pack(_hat_grid16(Y)).astype(F8)
            # bias row: img += Vb[r, h] * Ub[r, w] restores the 0.5 offset
            # (hat colsums are exactly 1 in-volume, 0 masked); row r=n lives
            # at [n % 2, n // 2, :]
            bias[n % 2, n // 2, 0:256] = (Y >= 0).astype(np.float32)
            bias[n % 2, n // 2, 256:512] = (np.float32(0.5)
                                            * (X >= 0).astype(np.float32))
        ray = T[b] - source[b, 0][None, None, :]
        raylen = np.sqrt((ray * ray).sum(-1)) / np.float32(S)  # [H, W]
        rs = np.empty((128, 512), np.float32)
        rs[:, 0:256] = raylen[0:128]
        rs[:, 256:512] = raylen[128:256]
        in_maps.append({"blob": blob, "bias": bias, "rs": rs})

    res = run_bass_kernel_spmd(nc, in_maps, core_ids=list(range(8)))
    out = np.zeros((2, 1, H, W), np.float32)
    for c in range(8):
        p = res.results[c]["partial"]
        out[core_batch[c], 0, 0:128] += p[:, 0:256]
        out[core_batch[c], 0, 128:256] += p[:, 256:512]
    return out
